# revision 1
# baseline (speedup 1.0000x reference)
"""Differential self-attention (B=2,T=2048,C=1024,H=16) on 8 trn2 NeuronCores.

Sharding: core c owns global heads {2c, 2c+1} for BOTH batches, and the
output shard (batch c//4, T-quarter c%4). Per core: column-parallel QKV
projections (fp32r matmuls), causal differential attention in S^T layout
(k on partitions, q on free; exp on ACT straight PSUM->SBUF; causal
masking via trapezoid-narrowed matmuls; softmax denominators via a
ones-column folded into the PV matmul; per-q normalization applied with
PE-replicated reciprocal rows), then an 8-core AllToAll redistributes
y^T head-shards into (batch, T-quarter) shards, and the core runs full
wo + compressor (wc/we) on its local T-quarter. Host only
slices/transposes inputs and concatenates outputs.
"""
import math
import sys

import numpy as np

for _p in ("/opt/trn_rl_repo", "/opt/trn_rl_repo/concourse"):
    if _p not in sys.path:
        sys.path.insert(0, _p)

import concourse.bass as bass  # noqa: E402
import concourse.tile as tile  # noqa: E402
from concourse import bacc, mybir  # noqa: E402
from concourse.bass_utils import run_bass_kernel_spmd  # noqa: E402

B, T, C, H = 2, 2048, 1024, 16
DH = C // H  # 64
N_LAYER = 12
LAMBDA_INIT = 0.8 - 0.6 * math.exp(-0.3 * (N_LAYER - 1))
SCALE = 1.0 / math.sqrt(DH)

TQ = 512        # q tile (free dim)
KBS = 128       # k block (partition dim)
NQT = T // TQ   # 4
NKB = T // KBS  # 16
TC = 512        # xT streaming chunk (T columns per chunk)
NTC = T // TC   # 4

F32 = mybir.dt.float32
F32R = mybir.dt.float32r
EXP = mybir.ActivationFunctionType.Exp

_CACHE = {}


def _build():
    nc = bacc.Bacc("TRN2", target_bir_lowering=False, debug=False, num_devices=8)
    d = nc.dram_tensor
    xT0 = d("xT0", [C, T], F32R, kind="ExternalInput").ap()
    xT1 = d("xT1", [C, T], F32R, kind="ExternalInput").ap()
    wqT = d("wqT", [C, 256], F32R, kind="ExternalInput").ap()
    wkT = d("wkT", [C, 256], F32R, kind="ExternalInput").ap()
    wvP = d("wvP", [C, 256], F32R, kind="ExternalInput").ap()  # cols 0-127 real
    woT = d("woT", [C, C], F32R, kind="ExternalInput").ap()
    wcT = d("wcT", [C, 512], F32R, kind="ExternalInput").ap()
    weT = d("weT", [512, C], F32R, kind="ExternalInput").ap()
    bcT = d("bcT", [128, 4], F32, kind="ExternalInput").ap()
    beR = d("beR", [128, C], F32, kind="ExternalInput").ap()
    lv = d("lv", [1, 64], F32, kind="ExternalInput").ap()
    mk = d("mk", [128, 128], F32, kind="ExternalInput").ap()
    out = d("out", [TQ, C], F32, kind="ExternalOutput").ap()

    r3 = lambda ap: ap.rearrange("(ko p) m -> p ko m", p=128)  # noqa: E731
    with tile.TileContext(nc) as tc:
        _emit(nc, tc, (r3(xT0), r3(xT1)), r3(wqT), r3(wkT), r3(wvP),
              r3(woT), r3(wcT), r3(weT), bcT, beR, lv, mk, out)
    nc.compile()
    return nc


def _emit(nc, tc, xT3, wqT3, wkT3, wvP3, woT3, wcT3, weT3, bcT, beR, lv, mk, out):
    from contextlib import ExitStack

    ctx = ExitStack()
    with ctx:
        const = ctx.enter_context(tc.tile_pool(name="const", bufs=1))
        tailw = ctx.enter_context(tc.tile_pool(name="tailw", bufs=1))
        attn_ctx = ctx.enter_context(ExitStack())
        qk = attn_ctx.enter_context(tc.tile_pool(name="qk", bufs=1))
        vpool = attn_ctx.enter_context(tc.tile_pool(name="vpool", bufs=1))
        dram = ctx.enter_context(tc.tile_pool(name="dram", bufs=1, space="DRAM"))

        # ---- lam = exp(sum(lq1*lk1)) - exp(sum(lq2*lk2)) + LAMBDA_INIT ----
        lv_sb = const.tile([1, 64], F32)
        nc.sync.dma_start(lv_sb[:], lv)
        ll = const.tile([1, 32], F32)
        nc.vector.tensor_mul(ll[:, 0:16], lv_sb[:, 0:16], lv_sb[:, 16:32])
        nc.vector.tensor_mul(ll[:, 16:32], lv_sb[:, 32:48], lv_sb[:, 48:64])
        ss = const.tile([1, 2], F32)
        nc.vector.reduce_sum(ss[:, 0:1], ll[:, None, 0:16], axis=mybir.AxisListType.X)
        nc.vector.reduce_sum(ss[:, 1:2], ll[:, None, 16:32], axis=mybir.AxisListType.X)
        es = const.tile([1, 2], F32)
        nc.scalar.activation(es[:], ss[:], EXP)  # loads exp table early too
        lam = const.tile([1, 1], F32)
        nc.vector.tensor_sub(lam[:], es[:, 0:1], es[:, 1:2])
        nc.vector.tensor_scalar_add(lam[:], lam[:], LAMBDA_INIT)
        # ones/lam rows to replicate r1 / lam*r2 onto 64 partitions via K=1 matmuls
        ones_f = const.tile([1, 64], F32)
        nc.vector.memset(ones_f[:], 1.0)
        onesr = const.tile([1, 64], F32R)
        nc.vector.tensor_copy(onesr[:], ones_f[:])
        lam_row = const.tile([1, 64], F32)
        nc.vector.tensor_copy(lam_row[:], lam[0:1, 0:1].to_broadcast([1, 64]))
        lamones = const.tile([1, 64], F32R)
        nc.vector.tensor_copy(lamones[:], lam_row[:])
        mk_f = const.tile([128, 128], F32)
        nc.sync.dma_start(mk_f[:], mk)
        mk_r = const.tile([128, 128], F32R)
        nc.vector.tensor_copy(mk_r[:], mk_f[:])
        be_sb = const.tile([128, C], F32)
        bc_sb = const.tile([128, 4], F32)
        nc.sync.dma_start(be_sb[:], beR)
        nc.sync.dma_start(bc_sb[:], bcT)

        # ---- persistent big tiles ----
        BF16 = mybir.dt.bfloat16
        qT1 = qk.tile([128, 2, T], BF16)   # [2heads x 64, batch, T]
        qT2 = qk.tile([128, 2, T], BF16)
        kT1 = qk.tile([128, 2, T], BF16)
        kT2 = qk.tile([128, 2, T], BF16)
        v5 = vpool.tile([128, NKB, 2, 2, 65], F32R)  # [Tmod, Tdiv, batch, head, dh+1]
        ones128 = const.tile([128, 1], F32)
        nc.vector.memset(ones128[:], 1.0)
        nc.vector.tensor_copy(
            v5[:, :, :, :, 64:65],
            ones128[:, 0:1, None, None, None].to_broadcast([128, NKB, 2, 2, 1]),
        )
        a2a_in = [dram.tile([8, 64, TQ], F32R, name=f"a2ain{h}") for h in range(2)]
        a2a_out = [dram.tile([8, 64, TQ], F32R, name=f"a2aout{h}") for h in range(2)]

        # ---- phase 1: QKV projections, streaming xT chunks ----
        with tc.tile_pool(name="projw", bufs=1) as projw, \
             tc.tile_pool(name="xs", bufs=2) as xs, \
             tc.tile_pool(name="pproj", bufs=3, space="PSUM") as pproj:
            wq_sb = projw.tile([128, 8, 256], F32R)
            wk_sb = projw.tile([128, 8, 256], F32R)
            wv_sb = projw.tile([128, 8, 256], F32R)
            nc.gpsimd.dma_start(wq_sb[:], wqT3)
            nc.gpsimd.dma_start(wk_sb[:], wkT3)
            nc.gpsimd.dma_start(wv_sb[:], wvP3)
            for b2 in range(2):
                for tcn in range(NTC):
                    xc = xs.tile([128, 8, TC], F32R, tag="xc", name="xc")
                    nc.sync.dma_start(xc[:], xT3[b2][:, :, bass.ts(tcn, TC)])
                    for w_sb, dst1, dst2 in ((wq_sb, qT1, qT2), (wk_sb, kT1, kT2)):
                        for m in range(2):
                            ps = pproj.tile([128, TC], F32, tag="pqk", name="ps_qk")
                            for k in range(8):
                                nc.tensor.matmul(ps[:], w_sb[:, k, bass.ts(m, 128)],
                                                 xc[:, k, :], start=(k == 0), stop=(k == 7))
                            dst = dst1 if m == 0 else dst2
                            nc.vector.tensor_copy(dst[:, b2, bass.ts(tcn, TC)], ps[:])
                    for tt in range(TC // 128):
                        psv = pproj.tile([128, 256], F32, tag="pv", name="ps_v")
                        for k in range(8):
                            nc.tensor.matmul(psv[:], xc[:, k, bass.ts(tt, 128)],
                                             wv_sb[:, k, :], start=(k == 0), stop=(k == 7))
                        nc.vector.tensor_copy(
                            v5[:, tcn * 4 + tt, b2, :, 0:64],
                            psv[:, 0:128].rearrange("p (h d) -> p h d", h=2),
                        )

        # ---- phase 2: attention (wo prefetches meanwhile) ----
        wo_sb = tailw.tile([128, 8, C], F32R)
        nc.sync.dma_start(wo_sb[:], woT3)
        ypool = attn_ctx.enter_context(tc.tile_pool(name="ypool", bufs=1))
        yT = ypool.tile([128, 2, T], F32R)  # [2heads x 64, batch, T]

        with tc.tile_pool(name="epool", bufs=2) as epool, \
             tc.tile_pool(name="cmb", bufs=2) as cmb, \
             tc.tile_pool(name="psS", bufs=2, space="PSUM") as psS, \
             tc.tile_pool(name="psU", bufs=1, space="PSUM") as psU, \
             tc.tile_pool(name="psR", bufs=1, space="PSUM") as psR:
            for h2 in range(2):
                hb = 64 * h2
                for b2 in range(2):
                    for qt in range(NQT):
                        kmax = 4 * qt + 4
                        us = []
                        for a, (qTa, kTa) in enumerate(((qT1, kT1), (qT2, kT2))):
                            u = psU.tile([65, TQ], F32, tag=f"U{a}", name=f"u{a}")
                            for kb0 in range(0, kmax, 2):
                                st = psS.tile([128, 2, TQ], F32, tag="S", name="st")
                                e = epool.tile([128, 2, TQ], F32R, tag="E", name="et")
                                offs = []
                                for i in (0, 1):
                                    kb = kb0 + i
                                    qo = max(0, (kb - 4 * qt) * 128)
                                    offs.append(qo)
                                    nc.tensor.matmul(
                                        st[:, i, qo:],
                                        kTa[hb:hb + 64, b2, bass.ts(kb, KBS)],
                                        qTa[hb:hb + 64, b2, qt * TQ + qo:(qt + 1) * TQ],
                                        start=True, stop=True)
                                mo = min(offs)
                                nc.scalar.activation(e[:, :, mo:], st[:, :, mo:],
                                                     EXP, scale=SCALE)
                                for i in (0, 1):
                                    kb, qo = kb0 + i, offs[i]
                                    if kb >= 4 * qt:  # diagonal: staircase mask
                                        nc.vector.tensor_mul(e[:, i, qo:qo + 128],
                                                             e[:, i, qo:qo + 128],
                                                             mk_r[:])
                                    nc.tensor.matmul(
                                        u[:, qo:], v5[:, kb, b2, h2, :], e[:, i, qo:],
                                        start=(kb == 0), stop=(kb == kmax - 1),
                                        skip_group_check=True)
                            us.append(u)
                        # combine: yT[:, qt] = U1/Z1 - lam*U2/Z2
                        zr1 = cmb.tile([1, TQ], F32, tag="zr1", name="zr1")
                        zr2 = cmb.tile([1, TQ], F32, tag="zr2", name="zr2")
                        nc.scalar.copy(zr1[:], us[0][64:65, :])
                        nc.scalar.copy(zr2[:], us[1][64:65, :])
                        rzf = cmb.tile([1, 2 * TQ], F32, tag="rzf", name="rzf")
                        nc.vector.reciprocal_approx_fast(rzf[:, 0:TQ], zr1[:])
                        nc.vector.reciprocal_approx_fast(rzf[:, TQ:], zr2[:])
                        rz = cmb.tile([1, 2 * TQ], F32R, tag="rz", name="rz")
                        nc.vector.tensor_copy(rz[:], rzf[:])
                        rb1 = psR.tile([64, TQ], F32, tag="rb1", name="rb1")
                        rb2 = psR.tile([64, TQ], F32, tag="rb2", name="rb2")
                        nc.tensor.matmul(rb1[:], onesr[:], rz[:, 0:TQ], start=True, stop=True)
                        nc.tensor.matmul(rb2[:], lamones[:], rz[:, TQ:], start=True, stop=True)
                        rbs = cmb.tile([128, TQ], F32, tag="rbs", name="rbs")
                        nc.scalar.copy(rbs[0:64, :], rb1[:])
                        nc.scalar.copy(rbs[64:128, :], rb2[:])
                        t1 = cmb.tile([64, TQ], F32, tag="t1", name="t1")
                        t2 = cmb.tile([64, TQ], F32, tag="t2", name="t2")
                        nc.vector.tensor_mul(t1[:], us[0][0:64, :], rbs[0:64, :])
                        nc.vector.tensor_mul(t2[:], us[1][0:64, :], rbs[64:128, :])
                        nc.vector.tensor_sub(yT[hb:hb + 64, b2, bass.ts(qt, TQ)],
                                             t1[:], t2[:])
                    # ship (h2, b2) yT rows into the A2A input shards
                    nc.sync.dma_start(
                        a2a_in[h2][b2 * 4:(b2 + 1) * 4, :, :].rearrange("j p t -> p j t"),
                        yT[hb:hb + 64, b2, :].rearrange("p (j t) -> p j t", t=TQ),
                    )
                nc.gpsimd.collective_compute(
                    "AllToAll", mybir.AluOpType.bypass,
                    replica_groups=[list(range(8))],
                    ins=[a2a_in[h2].opt()], outs=[a2a_out[h2].opt()],
                )

        # ---- phase 3: wo + compressor on the local (batch, T-quarter) ----
        attn_ctx.close()
        with tc.tile_pool(name="tail", bufs=1) as tail, \
             tc.tile_pool(name="opool", bufs=2) as opool, \
             tc.tile_pool(name="psT", bufs=2, space="PSUM") as psT:
            wc_sb = tail.tile([128, 8, 512], F32R)
            we_sb = tail.tile([128, 4, C], F32R)
            nc.sync.dma_start(wc_sb[:], wcT3)
            nc.sync.dma_start(we_sb[:], weT3)
            yf = tail.tile([128, 8, TQ], F32R)
            for kc in range(8):
                for h2 in range(2):
                    nc.sync.dma_start(yf[h2 * 64:(h2 + 1) * 64, kc, :],
                                      a2a_out[h2][kc, :, :])
            zT = tail.tile([128, 8, TQ], F32R)
            for mt in range(8):
                pz = psT.tile([128, TQ], F32, tag="pz", name="pz")
                for kc in range(8):
                    nc.tensor.matmul(pz[:], wo_sb[:, kc, bass.ts(mt, 128)], yf[:, kc, :],
                                     start=(kc == 0), stop=(kc == 7))
                nc.vector.tensor_copy(zT[:, mt, :], pz[:])
            hT = tail.tile([128, 4, TQ], F32R)
            for mt in range(4):
                ph = psT.tile([128, TQ], F32, tag="ph", name="ph")
                for kc in range(8):
                    nc.tensor.matmul(ph[:], wc_sb[:, kc, bass.ts(mt, 128)], zT[:, kc, :],
                                     start=(kc == 0), stop=(kc == 7))
                nc.scalar.add(hT[:, mt, :], ph[:], bc_sb[:, mt:mt + 1])
            for tt in range(4):
                o = opool.tile([128, C], F32, tag="o", name="o")
                for half in range(2):
                    po = psT.tile([128, TQ], F32, tag="po", name="po")
                    for kc in range(4):
                        nc.tensor.matmul(po[:], hT[:, kc, bass.ts(tt, 128)],
                                         we_sb[:, kc, bass.ts(half, TQ)],
                                         start=(kc == 0), stop=(kc == 3))
                    nc.vector.tensor_add(o[:, bass.ts(half, TQ)], po[:],
                                         be_sb[:, bass.ts(half, TQ)])
                nc.sync.dma_start(out[bass.ts(tt, 128), :], o[:])


def _prep_inputs(inputs):
    g = {k: np.asarray(v, dtype=np.float32) for k, v in inputs.items()}
    x, wq, wk, wv, wo = g["x"], g["wq"], g["wk"], g["wv"], g["wo"]
    wc, bc, we, be = g["wc"], g["bc"], g["we"], g["be"]
    lv = np.concatenate([g["lq1"], g["lk1"], g["lq2"], g["lk2"]]).reshape(1, 64).astype(np.float32)
    mk = np.ascontiguousarray(np.tril(np.ones((128, 128), np.float32)).T)
    woT = np.ascontiguousarray(wo.T)
    wcT = np.ascontiguousarray(wc.T)
    weT = np.ascontiguousarray(we.T)
    bcT = np.ascontiguousarray(bc.reshape(4, 128).T)
    beR = np.ascontiguousarray(np.broadcast_to(be[None, :], (128, C)))
    xT0 = np.ascontiguousarray(x[0].T)
    xT1 = np.ascontiguousarray(x[1].T)
    in_maps = []
    for c in range(8):
        r0, r1 = c * 128, (c + 1) * 128
        wqs = np.ascontiguousarray(np.concatenate([wq[r0:r1], wq[C + r0:C + r1]], 0).T)
        wks = np.ascontiguousarray(np.concatenate([wk[r0:r1], wk[C + r0:C + r1]], 0).T)
        wvs = np.zeros((C, 256), dtype=np.float32)
        wvs[:, 0:128] = wv[r0:r1].T
        in_maps.append({
            "xT0": xT0, "xT1": xT1, "wqT": wqs, "wkT": wks,
            "wvP": np.ascontiguousarray(wvs), "woT": woT,
            "wcT": wcT, "weT": weT, "bcT": bcT, "beR": beR, "lv": lv, "mk": mk,
        })
    return in_maps


def _run(inputs, trace=False, trace_cores=None):
    if "nc" not in _CACHE:
        _CACHE["nc"] = _build()
    in_maps = _prep_inputs(inputs)
    r = run_bass_kernel_spmd(
        _CACHE["nc"], in_maps, core_ids=list(range(8)), trace=trace,
        trace_cores=trace_cores,
    )
    o = np.empty((B, T, C), dtype=np.float32)
    for c in range(8):
        b, hg = c // 4, c % 4
        o[b, hg * TQ:(hg + 1) * TQ, :] = r.results[c]["out"]
    return o, r


def kernel(**inputs) -> np.ndarray:
    o, _ = _run(inputs, trace=False)
    return o



# revision 12
# speedup vs baseline: 1.2895x; 1.2895x over previous
"""Differential self-attention (B=2,T=2048,C=1024,H=16) on 8 trn2 NeuronCores.

Sharding: core c owns global heads {2c, 2c+1} for BOTH batches, and the
output shard (batch c//4, T-quarter c%4).

v3 design: bf16 operands throughout (fp32 PSUM). Per-head [q1;q2]/[k1;k2]
partition stacking makes the two attention streams' QK matmuls concurrent
64-row tile_position pairs. Causal staircase applied additively on PSUM
pre-exp (diag blocks emitted FIRST within each q-tile so the DVE mask
never stalls ScalarE). exp -> bf16 e, PV bf16 with a ones-column folded
in for softmax denominators. Projections are split per-head and emission-
interleaved with attention (head-0 proj feeds phases A/B, head-1 proj
fills phases C/D) so the PE stays dense and HAM never re-throttles.
Reciprocal-normalize combine is split: its PE part is deferred behind the
next q-tile's first QK pair so ScalarE keeps streaming exp. AllToAll per
h2 fires at the half-way point (after phases A/B) and at the end.
"""
import math
import sys

import numpy as np

for _p in ("/opt/trn_rl_repo", "/opt/trn_rl_repo/concourse"):
    if _p not in sys.path:
        sys.path.insert(0, _p)

import concourse.bass as bass  # noqa: E402
import concourse.tile as tile  # noqa: E402
from concourse import bacc, mybir  # noqa: E402
from concourse.bass_utils import run_bass_kernel_spmd  # noqa: E402

B, T, C, H = 2, 2048, 1024, 16
DH = C // H  # 64
N_LAYER = 12
LAMBDA_INIT = 0.8 - 0.6 * math.exp(-0.3 * (N_LAYER - 1))
SCALE = 1.0 / math.sqrt(DH)

TQ = 512        # q tile (free dim)
KBS = 128       # k block (partition dim)
NQT = T // TQ   # 4
NKB = T // KBS  # 16
TC = 512        # xT streaming chunk (T columns per chunk)
NTC = T // TC   # 4

F32 = mybir.dt.float32
F32R = mybir.dt.float32r
BF16 = mybir.dt.bfloat16
EXP = mybir.ActivationFunctionType.Exp

_CACHE = {}


def _build():
    nc = bacc.Bacc("TRN2", target_bir_lowering=False, debug=False, num_devices=8)
    d = nc.dram_tensor
    xT0 = d("xT0", [C, T], BF16, kind="ExternalInput").ap()
    xT1 = d("xT1", [C, T], BF16, kind="ExternalInput").ap()
    wqT = d("wqT", [C, 256], BF16, kind="ExternalInput").ap()
    wkT = d("wkT", [C, 256], BF16, kind="ExternalInput").ap()
    wvT = d("wvT", [C, 128], BF16, kind="ExternalInput").ap()
    woT = d("woT", [C, C], BF16, kind="ExternalInput").ap()
    wcT = d("wcT", [C, 512], BF16, kind="ExternalInput").ap()
    weT = d("weT", [512, C], BF16, kind="ExternalInput").ap()
    bcT = d("bcT", [128, 4], F32, kind="ExternalInput").ap()
    beR = d("beR", [128, C], F32, kind="ExternalInput").ap()
    lv = d("lv", [1, 64], F32, kind="ExternalInput").ap()
    mka = d("mka", [128, 128], F32, kind="ExternalInput").ap()
    out = d("out", [TQ, C], F32, kind="ExternalOutput").ap()

    r3 = lambda ap: ap.rearrange("(ko p) m -> p ko m", p=128)  # noqa: E731
    with tile.TileContext(nc) as tc:
        _emit(nc, tc, (r3(xT0), r3(xT1)), r3(wqT), r3(wkT), r3(wvT),
              r3(woT), r3(wcT), r3(weT), bcT, beR, lv, mka, out)
    nc.compile()
    return nc


def _emit(nc, tc, xT3, wqT3, wkT3, wvT3, woT3, wcT3, weT3, bcT, beR, lv, mka, out):
    from contextlib import ExitStack

    ctx = ExitStack()
    with ctx:
        const = ctx.enter_context(tc.tile_pool(name="const", bufs=1))
        tailw = ctx.enter_context(tc.tile_pool(name="tailw", bufs=1))
        attn_ctx = ctx.enter_context(ExitStack())
        qk = attn_ctx.enter_context(tc.tile_pool(name="qk", bufs=1))
        vpool = attn_ctx.enter_context(tc.tile_pool(name="vpool", bufs=1))
        projw = attn_ctx.enter_context(tc.tile_pool(name="projw", bufs=1))
        xs = attn_ctx.enter_context(tc.tile_pool(name="xs", bufs=2))
        epool = attn_ctx.enter_context(tc.tile_pool(name="epool", bufs=2))
        cmb = attn_ctx.enter_context(tc.tile_pool(name="cmb", bufs=2))
        ypool = attn_ctx.enter_context(tc.tile_pool(name="ypool", bufs=1))
        psS = attn_ctx.enter_context(tc.tile_pool(name="psS", bufs=2, space="PSUM"))
        psU = attn_ctx.enter_context(tc.tile_pool(name="psU", bufs=1, space="PSUM"))
        # shared 2-bank pool: proj accumulators and the per-qt reciprocal
        # broadcast tiles rotate through one tag (WAR-serialized by Tile)
        psM = attn_ctx.enter_context(tc.tile_pool(name="psM", bufs=2, space="PSUM"))
        dram = ctx.enter_context(tc.tile_pool(name="dram", bufs=1, space="DRAM"))

        # ---- projection weights first (small, needed immediately) ----
        wq_sb = projw.tile([128, 8, 256], BF16)
        wk_sb = projw.tile([128, 8, 256], BF16)
        wv_sb = projw.tile([128, 8, 128], BF16)
        nc.gpsimd.dma_start(wq_sb[:], wqT3)
        nc.gpsimd.dma_start(wk_sb[:], wkT3)
        nc.gpsimd.dma_start(wv_sb[:], wvT3)

        # ---- lam = exp(sum(lq1*lk1)) - exp(sum(lq2*lk2)) + LAMBDA_INIT ----
        lv_sb = const.tile([1, 64], F32)
        nc.sync.dma_start(lv_sb[:], lv)
        mka_sb = const.tile([128, 128], F32)
        nc.sync.dma_start(mka_sb[:], mka)
        ll = const.tile([1, 32], F32)
        nc.vector.tensor_mul(ll[:, 0:16], lv_sb[:, 0:16], lv_sb[:, 16:32])
        nc.vector.tensor_mul(ll[:, 16:32], lv_sb[:, 32:48], lv_sb[:, 48:64])
        ss = const.tile([1, 2], F32)
        nc.vector.reduce_sum(ss[:, 0:1], ll[:, None, 0:16], axis=mybir.AxisListType.X)
        nc.vector.reduce_sum(ss[:, 1:2], ll[:, None, 16:32], axis=mybir.AxisListType.X)
        es = const.tile([1, 2], F32)
        nc.scalar.activation(es[:], ss[:], EXP)  # loads exp table early too
        lam = const.tile([1, 1], F32)
        nc.vector.tensor_sub(lam[:], es[:, 0:1], es[:, 1:2])
        nc.vector.tensor_scalar_add(lam[:], lam[:], LAMBDA_INIT)
        # reciprocal-broadcast stationaries: rb rows 0-63 get rz1 (via st_a),
        # rows 64-127 get lam*rz2 (via st_b), two accumulating K=1 matmuls
        stf = const.tile([1, 2, 128], F32)
        nc.vector.memset(stf[:], 0.0)
        nc.vector.memset(stf[0:1, 0, 0:64], 1.0)
        nc.vector.tensor_copy(stf[0:1, 1, 64:128], lam[0:1, 0:1].to_broadcast([1, 64]))
        st2 = const.tile([1, 2, 128], F32R)
        nc.vector.tensor_copy(st2[:], stf[:])

        # ---- persistent big tiles ----
        # partition dim stacks the two attention streams: rows 0-63 = q1/k1,
        # rows 64-127 = q2/k2 (per head) -> concurrent 64-row QK matmul pairs
        qH = qk.tile([128, 2, 2, T], BF16)   # [2a x 64, head, batch, T]
        kH = qk.tile([128, 2, 2, T], BF16)
        v5 = vpool.tile([128, NKB, 2, 2, 65], BF16)  # [Tmod, Tdiv, batch, head, dh+1]
        ones128 = const.tile([128, 1], F32)
        nc.vector.memset(ones128[:], 1.0)
        nc.vector.tensor_copy(
            v5[:, :, :, :, 64:65],
            ones128[:, 0:1, None, None, None].to_broadcast([128, NKB, 2, 2, 1]),
        )
        yT = [ypool.tile([64, 2, T], BF16, name=f"yT{h}") for h in range(2)]
        a2a_in = [dram.tile([8, 64, TQ], BF16, name=f"a2ain{h}") for h in range(2)]
        a2a_out = [dram.tile([8, 64, TQ], BF16, name=f"a2aout{h}") for h in range(2)]

        # ---------- emission units ----------
        def proj_units(b2, part):
            """part 0: x-chunk DMA + head-0 q/k + both-head v (phases A/B).
            part 1: x-chunk re-DMA + head-1 q/k (phases C/D)."""
            for tcn in range(NTC):
                xc_holder = []

                def unit0(tcn=tcn, b2=b2, xh=xc_holder):
                    xc = xs.tile([128, 8, TC], BF16, tag="xc", name="xc")
                    nc.sync.dma_start(xc[:], xT3[b2][:, :, bass.ts(tcn, TC)])
                    xh.append(xc)
                yield unit0
                h = part
                for w_sb, dst in ((wq_sb, qH), (wk_sb, kH)):
                    def unit(h=h, w_sb=w_sb, dst=dst, tcn=tcn, b2=b2, xh=xc_holder):
                        xc = xh[0]
                        ps = psM.tile([128, TC], F32, tag="mb", name="ps_qk")
                        for k in range(8):
                            nc.tensor.matmul(ps[:], w_sb[:, k, bass.ts(h, 128)],
                                             xc[:, k, :], start=(k == 0),
                                             stop=(k == 7))
                        nc.vector.tensor_copy(dst[:, h, b2, bass.ts(tcn, TC)], ps[:])
                    yield unit
                if part == 0:
                    def unitv(tcn=tcn, b2=b2, xh=xc_holder):
                        xc = xh[0]
                        for tt in range(4):
                            psv = psM.tile([128, 128], F32, tag="mb", name="ps_v")
                            for k in range(8):
                                nc.tensor.matmul(psv[:], xc[:, k, bass.ts(tt, 128)],
                                                 wv_sb[:, k, :], start=(k == 0),
                                                 stop=(k == 7))
                            nc.vector.tensor_copy(
                                v5[:, tcn * 4 + tt, b2, :, 0:64],
                                psv[:].rearrange("p (h d) -> p h d", h=2),
                            )
                    yield unitv

        def attn_qt(h2, b2, qt):
            """Return (kb_units, combine_pre, combine_post)."""
            kmax = 4 * qt + 4
            # diag blocks first: their DVE mask-adds land while ScalarE is
            # still busy with the previous q-tile's exps
            order = list(range(4 * qt, kmax)) + list(range(0, 4 * qt))
            us_holder = []

            def kb_unit(idx, kb):
                if idx == 0:
                    us_holder.append([
                        psU.tile([65, TQ], F32, tag=f"u{a}", name=f"u{a}")
                        for a in range(2)
                    ])
                us = us_holder[0]
                qo = max(0, (kb - 4 * qt) * 128)
                S = psS.tile([128, 2, TQ], F32, tag="S", name="S")
                for a in range(2):
                    nc.tensor.matmul(
                        S[:, a, qo:],
                        kH[a * 64:(a + 1) * 64, h2, b2, bass.ts(kb, KBS)],
                        qH[a * 64:(a + 1) * 64, h2, b2, qt * TQ + qo:(qt + 1) * TQ],
                        start=True, stop=True)
                if kb >= 4 * qt:  # diagonal staircase: additive -1e9 pre-exp
                    nc.vector.tensor_add(
                        S[:, :, qo:qo + 128], S[:, :, qo:qo + 128],
                        mka_sb[:, None, :].to_broadcast([128, 2, 128]))
                e = epool.tile([128, 2, TQ], BF16, tag="E", name="e")
                nc.scalar.activation(e[:, :, qo:], S[:, :, qo:], EXP, scale=SCALE)
                for a in range(2):
                    nc.tensor.matmul(
                        us[a][:, qo:],
                        v5[:, kb, b2, h2, :],
                        e[:, a, qo:],
                        start=(idx == 0), stop=(idx == kmax - 1),
                        skip_group_check=True)

            units = [lambda idx=idx, kb=kb: kb_unit(idx, kb)
                     for idx, kb in enumerate(order)]

            def combine_pre():
                us = us_holder[0]
                zr = cmb.tile([1, 2, TQ], F32, tag="zr", name="zr")
                nc.vector.tensor_copy(zr[0:1, 0, :], us[0][64:65, :])
                nc.vector.tensor_copy(zr[0:1, 1, :], us[1][64:65, :])
                rzf = cmb.tile([1, 2, TQ], F32, tag="rzf", name="rzf")
                nc.vector.reciprocal_approx_fast(rzf[:], zr[:])
                rz = cmb.tile([1, 2, TQ], F32R, tag="rz", name="rz")
                nc.vector.tensor_copy(rz[:], rzf[:])
                us_holder.append(rz)

            def combine_post():
                us, rz = us_holder
                rb1 = psM.tile([64, TQ], F32, tag="mb", name="rb1")
                rb2 = psM.tile([64, TQ], F32, tag="mb", name="rb2")
                nc.tensor.matmul(rb1[:], st2[0:1, 0, 0:64], rz[0:1, 0, :],
                                 start=True, stop=True, skip_group_check=True)
                nc.tensor.matmul(rb2[:], st2[0:1, 1, 64:128], rz[0:1, 1, :],
                                 start=True, stop=True, skip_group_check=True)
                rbs = cmb.tile([128, TQ], F32, tag="rbs", name="rbs")
                nc.vector.tensor_copy(rbs[0:64, :], rb1[:])
                nc.vector.tensor_copy(rbs[64:128, :], rb2[:])
                t1 = cmb.tile([64, TQ], F32, tag="t1", name="t1")
                t2 = cmb.tile([64, TQ], F32, tag="t2", name="t2")
                nc.vector.tensor_mul(t1[:], us[0][0:64, :], rbs[0:64, :])
                nc.vector.tensor_mul(t2[:], us[1][0:64, :], rbs[64:128, :])
                nc.vector.tensor_sub(yT[h2][:, b2, bass.ts(qt, TQ)], t1[:], t2[:])

            return units, combine_pre, combine_post

        pending = []  # deferred combine_post closures

        def flush():
            while pending:
                pending.pop(0)()

        def run_qt(h2, b2, qt, proj_iter, n_proj):
            units, cpre, cpost = attn_qt(h2, b2, qt)
            emitted = 0
            for i, u in enumerate(units):
                u()
                if i == 1:
                    flush()
                if proj_iter is not None:
                    want = ((i + 1) * n_proj) // len(units)
                    while emitted < want:
                        try:
                            next(proj_iter)()
                            emitted += 1
                        except StopIteration:
                            proj_iter = None
                            break
            cpre()
            pending.append(cpost)

        def ship(h2, b2):
            nc.sync.dma_start(
                a2a_in[h2][b2 * 4:(b2 + 1) * 4, :, :].rearrange("j p t -> p j t"),
                yT[h2][:, b2, :].rearrange("p (j t) -> p j t", t=TQ),
            )

        def a2a(h2):
            nc.gpsimd.collective_compute(
                "AllToAll", mybir.AluOpType.bypass,
                replica_groups=[list(range(8))],
                ins=[a2a_in[h2].opt()], outs=[a2a_out[h2].opt()],
            )

        yf = tailw.tile([128, 8, TQ], BF16)
        wo_sb = tailw.tile([128, 8, C], BF16)
        wc_sb = tailw.tile([128, 8, 512], BF16)
        we_sb = tailw.tile([128, 4, C], BF16)
        be_sb = tailw.tile([128, C], F32)
        bc_sb = tailw.tile([128, 4], F32)

        # ---------- phase A: attn (h0, b0) pipelined with proj b0 part0 ----
        pjA = proj_units(0, 0)
        for _ in range(4):  # chunk 0 fully before qt0
            next(pjA)()
        for qt in range(NQT):
            run_qt(0, 0, qt, pjA if qt < 3 else None, 4)
        # tail weights prefetch (during phase B; gpsimd queue keeps the sync
        # queue free for x chunks and ships)
        nc.gpsimd.dma_start(wo_sb[:], woT3)

        # ---------- phase B: attn (h0, b1) pipelined with proj b1 part0 ----
        pjB = proj_units(1, 0)
        for _ in range(4):
            next(pjB)()
        for qt in range(NQT):
            run_qt(0, 1, qt, pjB if qt < 3 else None, 4)
        # head-0 shards complete after pending combine flushes inside C's qt0;
        # pre-emit the b0 ship here (yT[0][:,0] is final since phase A)
        ship(0, 0)

        # ---------- phase C: attn (h1, b0) with proj b0 part1 ----
        pjC = proj_units(0, 1)
        for _ in range(3):
            next(pjC)()
        first = True
        for qt in range(NQT):
            run_qt(1, 0, qt, pjC if qt < 3 else None, 3)
            if first:
                first = False
                ship(0, 1)
                a2a(0)
                # head-0 halves of y can land in SBUF during C/D
                for kc in range(8):
                    nc.gpsimd.dma_start(yf[0:64, kc, :], a2a_out[0][kc, :, :])
        nc.gpsimd.dma_start(wc_sb[:], wcT3)
        nc.gpsimd.dma_start(we_sb[:], weT3)
        nc.gpsimd.dma_start(be_sb[:], beR)
        nc.gpsimd.dma_start(bc_sb[:], bcT)
        flush()  # yT[1][:,0] qt3 must be emitted before its ship
        ship(1, 0)

        # ---------- phase D: attn (h1, b1) with proj b1 part1 ----
        pjD = proj_units(1, 1)
        for _ in range(3):
            next(pjD)()
        for qt in range(NQT):
            run_qt(1, 1, qt, pjD if qt < 3 else None, 3)
        flush()
        ship(1, 1)
        a2a(1)

        # ---------- tail: wo + compressor on the local (batch, T-quarter) ----
        attn_ctx.close()
        with tc.tile_pool(name="tail", bufs=1) as tail, \
             tc.tile_pool(name="opool", bufs=2) as opool, \
             tc.tile_pool(name="psT", bufs=2, space="PSUM") as psT:
            for kc in range(8):
                nc.sync.dma_start(yf[64:128, kc, :], a2a_out[1][kc, :, :])
            zT = tail.tile([128, 8, TQ], BF16)
            for mt in range(8):
                pz = psT.tile([128, TQ], F32, tag="pz", name="pz")
                for kc in range(8):
                    nc.tensor.matmul(pz[:], wo_sb[:, kc, bass.ts(mt, 128)], yf[:, kc, :],
                                     start=(kc == 0), stop=(kc == 7))
                nc.vector.tensor_copy(zT[:, mt, :], pz[:])
            hT = tail.tile([128, 4, TQ], BF16)
            for mt in range(4):
                ph = psT.tile([128, TQ], F32, tag="ph", name="ph")
                for kc in range(8):
                    nc.tensor.matmul(ph[:], wc_sb[:, kc, bass.ts(mt, 128)], zT[:, kc, :],
                                     start=(kc == 0), stop=(kc == 7))
                nc.scalar.add(hT[:, mt, :], ph[:], bc_sb[:, mt:mt + 1])
            for tt in range(4):
                o = opool.tile([128, C], F32, tag="o", name="o")
                for half in range(2):
                    po = psT.tile([128, TQ], F32, tag="po", name="po")
                    for kc in range(4):
                        nc.tensor.matmul(po[:], hT[:, kc, bass.ts(tt, 128)],
                                         we_sb[:, kc, bass.ts(half, TQ)],
                                         start=(kc == 0), stop=(kc == 3))
                    nc.vector.tensor_add(o[:, bass.ts(half, TQ)], po[:],
                                         be_sb[:, bass.ts(half, TQ)])
                nc.sync.dma_start(out[bass.ts(tt, 128), :], o[:])


def _prep_inputs(inputs):
    import ml_dtypes
    bf16 = ml_dtypes.bfloat16
    g = {k: np.asarray(v, dtype=np.float32) for k, v in inputs.items()}
    x, wq, wk, wv, wo = g["x"], g["wq"], g["wk"], g["wv"], g["wo"]
    wc, bc, we, be = g["wc"], g["bc"], g["we"], g["be"]
    lv = np.concatenate([g["lq1"], g["lk1"], g["lq2"], g["lk2"]]).reshape(1, 64).astype(np.float32)
    p, c_ = np.arange(128)[:, None], np.arange(128)[None, :]
    mka = np.where(c_ >= p, 0.0, -1e9).astype(np.float32)
    woT = np.ascontiguousarray(wo.T).astype(bf16)
    wcT = np.ascontiguousarray(wc.T).astype(bf16)
    weT = np.ascontiguousarray(we.T).astype(bf16)
    bcT = np.ascontiguousarray(bc.reshape(4, 128).T)
    beR = np.ascontiguousarray(np.broadcast_to(be[None, :], (128, C)))
    xT0 = np.ascontiguousarray(x[0].T).astype(bf16)
    xT1 = np.ascontiguousarray(x[1].T).astype(bf16)
    in_maps = []
    for c in range(8):
        h0, h1 = 2 * c, 2 * c + 1
        # per-head column layout: [q1_h0 | q2_h0 | q1_h1 | q2_h1]
        wqs = np.ascontiguousarray(np.concatenate(
            [wq[64 * h0:64 * h0 + 64], wq[C + 64 * h0:C + 64 * h0 + 64],
             wq[64 * h1:64 * h1 + 64], wq[C + 64 * h1:C + 64 * h1 + 64]], 0).T).astype(bf16)
        wks = np.ascontiguousarray(np.concatenate(
            [wk[64 * h0:64 * h0 + 64], wk[C + 64 * h0:C + 64 * h0 + 64],
             wk[64 * h1:64 * h1 + 64], wk[C + 64 * h1:C + 64 * h1 + 64]], 0).T).astype(bf16)
        wvs = np.ascontiguousarray(wv[128 * c:128 * (c + 1)].T).astype(bf16)
        in_maps.append({
            "xT0": xT0, "xT1": xT1, "wqT": wqs, "wkT": wks, "wvT": wvs,
            "woT": woT, "wcT": wcT, "weT": weT, "bcT": bcT, "beR": beR,
            "lv": lv, "mka": mka,
        })
    return in_maps


def _run(inputs, trace=False, trace_cores=None):
    if "nc" not in _CACHE:
        _CACHE["nc"] = _build()
    in_maps = _prep_inputs(inputs)
    r = run_bass_kernel_spmd(
        _CACHE["nc"], in_maps, core_ids=list(range(8)), trace=trace,
        trace_cores=trace_cores,
    )
    o = np.empty((B, T, C), dtype=np.float32)
    for c in range(8):
        b, hg = c // 4, c % 4
        o[b, hg * TQ:(hg + 1) * TQ, :] = r.results[c]["out"]
    return o, r


def kernel(**inputs) -> np.ndarray:
    o, _ = _run(inputs, trace=False)
    return o
